# revision 1
# baseline (speedup 1.0000x reference)
"""Criss-Cross Attention Trainium2 kernel.

Shapes (hardcoded): B=8, C=512, CQ=128, H=W=96. One image per NeuronCore
(data-parallel over batch across 8 cores); params replicated.

Math per image (reference):
  q = Wq@x+bq [128,HW]; k = Wk@x+bk; v = Wv@x+bv [512,HW]
  E_H[h,w,j] = q[:,h,w].k[:,j,w]  (diag j==h masked -inf)
  E_W[h,w,j] = q[:,h,w].k[:,h,j]
  att = softmax over concat(j in H | j in W)
  out = gamma*(sum_j v[:,j,w]att_H + sum_j v[:,h,j]att_W) + x

Kernel trick: with unnormalized exp-weights S, and Z the softmax denom,
  out = x + gamma/Z * (AV_H + AV_W) + gamma*bv
(bv passes through softmax exactly since weights sum to 1.)
"""

import os
import sys

for _p in ("/opt/trn_rl_repo", os.path.expanduser("~/.axon_site/_ro/trn_rl_repo")):
    if os.path.isdir(_p) and _p not in sys.path:
        sys.path.insert(0, _p)

import numpy as np

import concourse.bass as bass
import concourse.mybir as mybir
import concourse.tile as tile
from concourse import bacc
from concourse.bass_utils import run_bass_kernel_spmd

B, C, CQ, H, W = 8, 512, 128, 96, 96
N = H * W  # 9216 pixels
NT = 512  # pixel tile for projections
NTILES = N // NT  # 18
CCH = C // 128  # 4 chunks of channels
F32 = mybir.dt.float32
F32R = mybir.dt.float32r
BF16 = mybir.dt.bfloat16
NEG = -1.0e30


def _mm(nc, out, lhsT, rhs, start, stop, dt=None):
    if dt is not None:
        lhsT = lhsT.bitcast(dt)
        rhs = rhs.bitcast(dt)
    nc.tensor.matmul(out, lhsT, rhs, start=start, stop=stop)


def build_kernel():
    nc = bacc.Bacc("TRN2", target_bir_lowering=False, debug=False, num_devices=B)

    x_d = nc.dram_tensor("x", [C, N], F32R, kind="ExternalInput")
    wq_d = nc.dram_tensor("wqT", [C, CQ], F32R, kind="ExternalInput")
    wk_d = nc.dram_tensor("wkT", [C, CQ], F32R, kind="ExternalInput")
    wv_d = nc.dram_tensor("wvT", [C, C], F32R, kind="ExternalInput")
    bq_d = nc.dram_tensor("bq", [CQ, 1], F32, kind="ExternalInput")
    bk_d = nc.dram_tensor("bk", [CQ, 1], F32, kind="ExternalInput")
    gb_d = nc.dram_tensor("gb", [C, 1], F32, kind="ExternalInput")  # gamma*bv
    gcol_d = nc.dram_tensor("gcol", [H, 1], F32, kind="ExternalInput")  # gamma
    id32_d = nc.dram_tensor("id32", [128, 128], F32, kind="ExternalInput")
    id16_d = nc.dram_tensor("id16", [128, 128], BF16, kind="ExternalInput")
    nid_d = nc.dram_tensor("negid", [H, H], F32, kind="ExternalInput")  # -1e30*I

    out_d = nc.dram_tensor("out", [C, N], F32, kind="ExternalOutput")
    vt_d = nc.dram_tensor("vt_scratch", [N, C], BF16)
    ob_d = nc.dram_tensor("outb_scratch", [N, C], BF16)

    with tile.TileContext(nc) as tc:
        # ---------------- long-lived pools ----------------
        with (
            tc.tile_pool(name="att", bufs=1) as attp,
            tc.tile_pool(name="small", bufs=1) as smp,
            tc.tile_pool(name="zpsum", bufs=1, space="PSUM") as zpp,
        ):
            attH = attp.tile([H, W * H], BF16, tag="attH")  # [j, (w,h)]
            attW = attp.tile([H, H * W], BF16, tag="attW")  # [j, (h,w)]

            gcol = smp.tile([H, 1], F32, tag="gcol")
            id32 = smp.tile([128, 128], F32, tag="id32")
            id16 = smp.tile([128, 128], BF16, tag="id16")
            nid = smp.tile([H, H], F32, tag="negid")
            ones16 = smp.tile([H, 1], BF16, tag="ones16")
            gb_sb = smp.tile([128, CCH], F32, tag="gb")
            sw_sb = smp.tile([H, W], F32, tag="sw_sb")
            sh_sb = smp.tile([H, W], F32, tag="sh_sb")
            z_sb = smp.tile([H, W], F32, tag="z_sb")
            r_sb = smp.tile([H, W], F32, tag="r_sb")
            izg = smp.tile([H, W], F32, tag="izg")  # gamma/Z  [h,w]
            izgT = smp.tile([W, H], F32, tag="izgT")  # gamma/Z  [w,h]

            nc.sync.dma_start(out=gcol, in_=gcol_d[:])
            nc.sync.dma_start(out=id32, in_=id32_d[:])
            nc.sync.dma_start(out=id16, in_=id16_d[:])
            nc.sync.dma_start(out=nid, in_=nid_d[:])
            nc.sync.dma_start(
                out=gb_sb, in_=gb_d.rearrange("(cc p) one -> p (cc one)", p=128)
            )
            nc.vector.memset(ones16, 1.0)

            # softmax denominator accumulators (PSUM, whole phase 2)
            shp = zpp.tile([H, W], F32, tag="shp")  # sum_j expH -> [h,w]
            swp = zpp.tile([W, H], F32, tag="swp")  # sum_j expW -> [w,h]

            # ---------------- phases 1+2 (q,k live) ----------------
            _qk_cm = tc.tile_pool(name="qk", bufs=1)
            qkp = _qk_cm.__enter__()
            q_sb = qkp.tile([CQ, N], F32, tag="q")
            k_sb = qkp.tile([CQ, N], F32, tag="k")
            # ---------------- phase 1: projections ----------------
            with (
                tc.tile_pool(name="wts", bufs=1) as wp,
                tc.tile_pool(name="xin", bufs=2) as xp,
                tc.tile_pool(name="vstage", bufs=3) as vsp,
                tc.tile_pool(name="p1psum", bufs=2, space="PSUM") as pp1,
            ):
                wq = wp.tile([128, CCH, CQ], F32R, tag="wq")
                wk = wp.tile([128, CCH, CQ], F32R, tag="wk")
                wv = wp.tile([128, CCH, C], F32R, tag="wv")
                bq = wp.tile([CQ, 1], F32, tag="bq")
                bk = wp.tile([CQ, 1], F32, tag="bk")
                nc.sync.dma_start(
                    out=wq, in_=wq_d.rearrange("(cc p) o -> p cc o", p=128)
                )
                nc.sync.dma_start(
                    out=wk, in_=wk_d.rearrange("(cc p) o -> p cc o", p=128)
                )
                nc.sync.dma_start(
                    out=wv, in_=wv_d.rearrange("(cc p) o -> p cc o", p=128)
                )
                nc.sync.dma_start(out=bq, in_=bq_d[:])
                nc.sync.dma_start(out=bk, in_=bk_d[:])

                x_r = x_d.rearrange("(cc p) n -> p cc n", p=128)
                for nt in range(NTILES):
                    nsl = bass.ts(nt, NT)
                    xt = xp.tile([128, CCH, NT], F32R, tag="xt")
                    nc.sync.dma_start(out=xt, in_=x_r[:, :, nsl])

                    qp = pp1.tile([CQ, NT], F32, tag="qp")
                    kp = pp1.tile([CQ, NT], F32, tag="kp")
                    for cc in range(CCH):
                        _mm(nc, qp, wq[:, cc, :], xt[:, cc, :], cc == 0, cc == 3)
                    for cc in range(CCH):
                        _mm(nc, kp, wk[:, cc, :], xt[:, cc, :], cc == 0, cc == 3)
                    nc.vector.tensor_scalar_add(q_sb[:, nsl], qp, bq)
                    nc.vector.tensor_scalar_add(k_sb[:, nsl], kp, bk)
                    # vT tile: out[p=n128, c] = sum_cc x[cc,p_n].T @ wv[cc]
                    vs = vsp.tile([128, NT // 128, C], BF16, tag="vs")
                    for s in range(NT // 128):
                        vp = pp1.tile([128, C], F32, tag="vp")
                        for cc in range(CCH):
                            _mm(
                                nc, vp, xt[:, cc, bass.ts(s, 128)], wv[:, cc, :],
                                cc == 0, cc == 3,
                            )
                        nc.vector.tensor_copy(vs[:, s, :], vp)
                    nc.sync.dma_start(
                        out=vt_d.rearrange("(t s p) c -> t p s c", s=4, p=128)[nt],
                        in_=vs,
                    )

            # ---------------- phase 2: energies + softmax ----------------
            q3 = q_sb.rearrange("p (h w) -> p h w", w=W)
            k3 = k_sb.rearrange("p (h w) -> p h w", w=W)
            attH3 = attH.rearrange("p (w h) -> p w h", w=W)
            attW3 = attW.rearrange("p (h w) -> p h w", h=H)
            with tc.tile_pool(name="epsum", bufs=3, space="PSUM") as epp:
                for i in range(H):
                    # E_H^T[j,h] at w=i:  lhsT=k[:, :, i] (j on free), rhs=q[:, :, i]
                    eh = epp.tile([H, H], F32, tag="eh")
                    _mm(nc, eh, k3[:, :, i], q3[:, :, i], True, False)
                    _mm(nc, eh, id32[:H, :H], nid, False, True)
                    nc.scalar.activation(
                        attH3[:, i, :], eh, mybir.ActivationFunctionType.Exp
                    )
                    # E_W^T[j,w] at h=i:  lhsT=k[:, i, :], rhs=q[:, i, :]
                    ew = epp.tile([H, W], F32, tag="ew")
                    _mm(nc, ew, k3[:, i, :], q3[:, i, :], True, True)
                    nc.scalar.activation(
                        attW3[:, i, :], ew, mybir.ActivationFunctionType.Exp
                    )
                    # denom columns: ones^T reductions over j (partition dim)
                    _mm(
                        nc, shp[:, i : i + 1], attH3[:, i, :], ones16,
                        i == 0, i == H - 1,
                    )
                    _mm(
                        nc, swp[:, i : i + 1], attW3[:, i, :], ones16,
                        i == 0, i == H - 1,
                    )

                # Z = SH + SW^T ; izg = gamma/Z (both orientations)
                nc.scalar.copy(sw_sb, swp)
                swt = epp.tile([H, W], F32, tag="eh")
                nc.tensor.transpose(swt, sw_sb, id32[:H, :H])
                nc.scalar.copy(sh_sb, shp)
                nc.vector.tensor_add(z_sb, sh_sb, swt)
                nc.vector.reciprocal(r_sb, z_sb)
                nc.vector.tensor_scalar_mul(izg, r_sb, gcol)
                izgtp = epp.tile([W, H], F32, tag="ew")
                nc.tensor.transpose(izgtp, izg, id32[:H, :H])
                nc.scalar.copy(izgT, izgtp)

            _qk_cm.__exit__(None, None, None)

            # ---------------- phase 3: attention @ V ----------------
            vt_str = vt_d.rearrange("(j w) c -> w j c", w=W)
            ob_str = ob_d.rearrange("(h w) c -> w h c", w=W)
            _acc_cm = tc.tile_pool(name="acc", bufs=1)
            accp = _acc_cm.__enter__()
            accA = accp.tile([W, H * C], BF16, tag="accA")  # [w, (h,c)]
            accA3 = accA.rearrange("p (h c) -> p h c", c=C)
            G = 4
            va_r = vt_d.rearrange("(g i j) c -> g j i c", i=G, j=W)
            vb_r = vt_d.rearrange("(j g w) c -> g j w c", g=H // G, w=G)
            obw_r = ob_d.rearrange("(h g w) c -> g h w c", g=H // G, w=G)
            with (
                tc.tile_pool(name="vblk", bufs=3) as vbp,
                tc.tile_pool(name="ostage", bufs=3) as osp,
                tc.tile_pool(name="opsum", bufs=3, space="PSUM") as opp,
            ):
                for g in range(H // G):
                    va = vbp.tile([H, G, C], BF16, tag="va")
                    nc.sync.dma_start(out=va, in_=va_r[g])
                    vb = vbp.tile([H, G, C], BF16, tag="vb")
                    nc.sync.dma_start(out=vb, in_=vb_r[g])
                    obs = osp.tile([H, G, C], BF16, tag="ob")
                    for i2 in range(G):
                        i = g * G + i2
                        # pass A (row attention, fixed h=i): out[w,c] -> accA
                        oap = opp.tile([W, C], F32, tag="oap")
                        _mm(nc, oap, attW3[:, i, :], va[:, i2, :], True, True)
                        nc.vector.tensor_scalar_mul(
                            accA3[:, i, :], oap, izgT[:, i : i + 1]
                        )
                        # pass B (col attention, fixed w=i): out[h,c] -> DRAM
                        obp = opp.tile([H, C], F32, tag="obp")
                        _mm(nc, obp, attH3[:, i, :], vb[:, i2, :], True, True)
                        nc.scalar.activation(
                            obs[:, i2, :], obp,
                            mybir.ActivationFunctionType.Identity,
                            scale=izg[:, i : i + 1],
                        )
                    nc.scalar.dma_start(out=obw_r[g], in_=obs)

            # ---------------- phase 4: combine, transpose, +x ----------------
            HQ = 4  # h-rows per group
            FW = HQ * W  # 384 output columns per tile
            with (
                tc.tile_pool(name="fin", bufs=3) as fp_,
                tc.tile_pool(name="fsum", bufs=6) as fsp,
                tc.tile_pool(name="fout", bufs=3) as fop,
                tc.tile_pool(name="fpsum", bufs=4, space="PSUM") as fpp,
            ):
                tb_r = ob_d.rearrange("(g i w) c -> g w i c", i=HQ, w=W)
                x4_r = x_d.rearrange("(cc p) n -> p cc n", p=128)
                o4_r = out_d.rearrange("(cc p) n -> p cc n", p=128)
                for hq in range(H // HQ):
                    nsl = bass.ts(hq, FW)
                    tb = fp_.tile([W, HQ, C], BF16, tag="tb")
                    nc.sync.dma_start(out=tb, in_=tb_r[hq])
                    xt = fop.tile([128, CCH, FW], F32, tag="xt4")
                    nc.sync.dma_start(out=xt, in_=x4_r[:, :, nsl].bitcast(F32))
                    sus = []
                    for i2 in range(HQ):
                        h = hq * HQ + i2
                        su = fsp.tile([W, C], BF16, tag="su")
                        nc.vector.tensor_add(su, accA3[:, h, :], tb[:, i2, :])
                        sus.append(su)
                    ot = fop.tile([128, CCH, FW], F32, tag="ot")
                    for cc in range(CCH):
                        csl = bass.ts(cc, 128)
                        op = fpp.tile([128, FW], BF16, tag="op")
                        for i2 in range(HQ):
                            nc.tensor.matmul(
                                op[:, bass.ts(i2, W)], sus[i2][:, csl],
                                id16[:W, :W], is_transpose=True,
                                start=(i2 == 0), stop=(i2 == HQ - 1),
                            )
                        xb = fop.tile([128, FW], F32, tag="xb4")
                        nc.scalar.activation(
                            xb, op, mybir.ActivationFunctionType.Identity,
                            bias=gb_sb[:, cc : cc + 1], scale=1.0,
                        )
                        nc.vector.tensor_add(ot[:, cc, :], xb, xt[:, cc, :])
                    nc.scalar.dma_start(out=o4_r[:, :, nsl], in_=ot)
            _acc_cm.__exit__(None, None, None)

    nc.compile()
    return nc


_NC_CACHE = {}


def _get_nc():
    if "nc" not in _NC_CACHE:
        _NC_CACHE["nc"] = build_kernel()
    return _NC_CACHE["nc"]


def make_in_maps(x, Wq, bq, Wk, bk, Wv, bv, gamma):
    x = np.ascontiguousarray(np.asarray(x, np.float32))
    gamma = np.asarray(gamma, np.float32)
    wqT = np.ascontiguousarray(np.asarray(Wq, np.float32).T)
    wkT = np.ascontiguousarray(np.asarray(Wk, np.float32).T)
    wvT = np.ascontiguousarray(np.asarray(Wv, np.float32).T)
    import ml_dtypes

    shared = {
        "wqT": wqT,
        "wkT": wkT,
        "wvT": wvT,
        "bq": np.asarray(bq, np.float32).reshape(CQ, 1),
        "bk": np.asarray(bk, np.float32).reshape(CQ, 1),
        "gb": (gamma[0] * np.asarray(bv, np.float32)).reshape(C, 1),
        "gcol": np.full((H, 1), gamma[0], np.float32),
        "id32": np.eye(128, dtype=np.float32),
        "id16": np.eye(128, dtype=ml_dtypes.bfloat16),
        "negid": (NEG * np.eye(H)).astype(np.float32),
    }
    return [
        {**shared, "x": x[b].reshape(C, N)} for b in range(B)
    ]


def kernel(x, Wq, bq, Wk, bk, Wv, bv, gamma, _trace=False):
    nc = _get_nc()
    in_maps = make_in_maps(x, Wq, bq, Wk, bk, Wv, bv, gamma)
    res = run_bass_kernel_spmd(nc, in_maps, list(range(B)), trace=_trace)
    out = np.stack([res.results[b]["out"].reshape(C, H, W) for b in range(B)])
    return out.astype(np.float32)



# revision 10
# speedup vs baseline: 3.8314x; 3.8314x over previous
"""Criss-Cross Attention Trainium2 kernel.

Shapes (hardcoded): B=8, C=512, CQ=128, H=W=96. One image per NeuronCore
(data-parallel over batch across 8 cores); params replicated.

Math per image (reference):
  q = Wq@x+bq [128,HW]; k = Wk@x+bk; v = Wv@x+bv [512,HW]
  E_H[h,w,j] = q[:,h,w].k[:,j,w]  (diag j==h masked -inf)
  E_W[h,w,j] = q[:,h,w].k[:,h,j]
  att = softmax over concat(j in H | j in W)
  out = gamma*(sum_j v[:,j,w]att_H + sum_j v[:,h,j]att_W) + x

Kernel trick: with unnormalized exp-weights S, and Z the softmax denom,
  out = x + gamma/Z * (AV_H + AV_W) + gamma*bv
(bv passes through softmax exactly since weights sum to 1.)
"""

import os
import sys

for _p in ("/opt/trn_rl_repo", os.path.expanduser("~/.axon_site/_ro/trn_rl_repo")):
    if os.path.isdir(_p) and _p not in sys.path:
        sys.path.insert(0, _p)

import numpy as np

import concourse.bass as bass
import concourse.mybir as mybir
import concourse.tile as tile
from concourse import bacc
from concourse.bass_utils import run_bass_kernel_spmd

B, C, CQ, H, W = 8, 512, 128, 96, 96
N = H * W  # 9216 pixels
NT = 512  # pixel tile for projections
NTILES = N // NT  # 18
CCH = C // 128  # 4 chunks of channels
F32 = mybir.dt.float32
F32R = mybir.dt.float32r
BF16 = mybir.dt.bfloat16
NEG = -1.0e30


def _mm(nc, out, lhsT, rhs, start, stop, dt=None):
    if dt is not None:
        lhsT = lhsT.bitcast(dt)
        rhs = rhs.bitcast(dt)
    nc.tensor.matmul(out, lhsT, rhs, start=start, stop=stop)


def build_kernel():
    nc = bacc.Bacc("TRN2", target_bir_lowering=False, debug=False, num_devices=B)

    x_d = nc.dram_tensor("x", [C, N], BF16, kind="ExternalInput")
    wq_d = nc.dram_tensor("wqT", [C, CQ], BF16, kind="ExternalInput")
    wk_d = nc.dram_tensor("wkT", [C, CQ], BF16, kind="ExternalInput")
    wv_d = nc.dram_tensor("wvT", [C, C], BF16, kind="ExternalInput")
    bq_d = nc.dram_tensor("bq", [CQ, 1], F32, kind="ExternalInput")
    bk_d = nc.dram_tensor("bk", [CQ, 1], F32, kind="ExternalInput")
    gb_d = nc.dram_tensor("gb", [C, 1], F32, kind="ExternalInput")  # gamma*bv
    gcol_d = nc.dram_tensor("gcol", [H, 1], F32, kind="ExternalInput")  # gamma
    id32_d = nc.dram_tensor("id32", [128, 128], F32, kind="ExternalInput")
    id16_d = nc.dram_tensor("id16", [128, 128], BF16, kind="ExternalInput")
    nid_d = nc.dram_tensor("negid", [H, H], BF16, kind="ExternalInput")  # -1e30*I

    out_d = nc.dram_tensor("out", [C, N], BF16, kind="ExternalOutput")
    vt_d = nc.dram_tensor("vt_scratch", [N, C], BF16)
    ob_d = nc.dram_tensor("outb_scratch", [N, C], BF16)

    with tile.TileContext(nc) as tc:
        # ---------------- long-lived pools ----------------
        with (
            tc.tile_pool(name="att", bufs=1) as attp,
            tc.tile_pool(name="small", bufs=1) as smp,
            tc.tile_pool(name="zpsum", bufs=1, space="PSUM") as zpp,
        ):
            attH = attp.tile([H, W * H], BF16, tag="attH")  # [j, (w,h)]
            attW = attp.tile([H, H * W], BF16, tag="attW")  # [j, (h,w)]

            gcol = smp.tile([H, 1], F32, tag="gcol")
            id32 = smp.tile([128, 128], F32, tag="id32")
            id16 = smp.tile([128, 128], BF16, tag="id16")
            nid = smp.tile([H, H], BF16, tag="negid")
            ones16 = smp.tile([H, 1], BF16, tag="ones16")
            gb_sb = smp.tile([128, CCH], F32, tag="gb")
            sw_sb = smp.tile([H, W], F32, tag="sw_sb")
            sh_sb = smp.tile([H, W], F32, tag="sh_sb")
            z_sb = smp.tile([H, W], F32, tag="z_sb")
            r_sb = smp.tile([H, W], F32, tag="r_sb")
            izg = smp.tile([H, W], F32, tag="izg")  # gamma/Z  [h,w]
            izgT = smp.tile([W, H], F32, tag="izgT")  # gamma/Z  [w,h]

            nc.sync.dma_start(out=gcol, in_=gcol_d[:])
            nc.sync.dma_start(out=id32, in_=id32_d[:])
            nc.sync.dma_start(out=id16, in_=id16_d[:])
            nc.sync.dma_start(out=nid, in_=nid_d[:])
            nc.sync.dma_start(
                out=gb_sb, in_=gb_d.rearrange("(cc p) one -> p (cc one)", p=128)
            )
            nc.vector.memset(ones16, 1.0)

            # softmax denominator accumulators (PSUM, whole phase 2)
            shp = zpp.tile([H, W], F32, tag="shp")  # sum_j expH -> [h,w]
            swp = zpp.tile([W, H], F32, tag="swp")  # sum_j expW -> [w,h]

            # ---------------- phases 1+2 (q,k live) ----------------
            _qk_cm = tc.tile_pool(name="qk", bufs=1)
            qkp = _qk_cm.__enter__()
            q_sb = qkp.tile([CQ, N], BF16, tag="q")
            k_sb = qkp.tile([CQ, N], BF16, tag="k")
            # ---------------- phase 1: projections ----------------
            with (
                tc.tile_pool(name="wts", bufs=1) as wp,
                tc.tile_pool(name="xin", bufs=2) as xp,
                tc.tile_pool(name="vstage", bufs=3) as vsp,
                tc.tile_pool(name="p1psum", bufs=2, space="PSUM") as pp1,
            ):
                wq = wp.tile([128, CCH, CQ], BF16, tag="wq")
                wk = wp.tile([128, CCH, CQ], BF16, tag="wk")
                wv = wp.tile([128, CCH, C], BF16, tag="wv")
                bq = wp.tile([CQ, 1], F32, tag="bq")
                bk = wp.tile([CQ, 1], F32, tag="bk")
                nc.sync.dma_start(
                    out=wq, in_=wq_d.rearrange("(cc p) o -> p cc o", p=128)
                )
                nc.sync.dma_start(
                    out=wk, in_=wk_d.rearrange("(cc p) o -> p cc o", p=128)
                )
                nc.sync.dma_start(
                    out=wv, in_=wv_d.rearrange("(cc p) o -> p cc o", p=128)
                )
                nc.sync.dma_start(out=bq, in_=bq_d[:])
                nc.sync.dma_start(out=bk, in_=bk_d[:])

                x_r = x_d.rearrange("(cc p) n -> p cc n", p=128)
                for nt in range(NTILES):
                    nsl = bass.ts(nt, NT)
                    xt = xp.tile([128, CCH, NT], BF16, tag="xt")
                    nc.sync.dma_start(out=xt, in_=x_r[:, :, nsl])

                    qp = pp1.tile([CQ, NT], F32, tag="qp")
                    kp = pp1.tile([CQ, NT], F32, tag="kp")
                    for cc in range(CCH):
                        _mm(nc, qp, wq[:, cc, :], xt[:, cc, :], cc == 0, cc == 3)
                    for cc in range(CCH):
                        _mm(nc, kp, wk[:, cc, :], xt[:, cc, :], cc == 0, cc == 3)
                    nc.vector.tensor_scalar_add(q_sb[:, nsl], qp, bq)
                    nc.vector.tensor_scalar_add(k_sb[:, nsl], kp, bk)
                    # vT tile: out[p=n128, c] = sum_cc x[cc,p_n].T @ wv[cc]
                    vs = vsp.tile([128, NT // 128, C], BF16, tag="vs")
                    for s in range(NT // 128):
                        vp = pp1.tile([128, C], F32, tag="vp")
                        for cc in range(CCH):
                            _mm(
                                nc, vp, xt[:, cc, bass.ts(s, 128)], wv[:, cc, :],
                                cc == 0, cc == 3,
                            )
                        nc.vector.tensor_copy(vs[:, s, :], vp)
                    nc.sync.dma_start(
                        out=vt_d.rearrange("(t s p) c -> t p s c", s=4, p=128)[nt],
                        in_=vs,
                    )

            # ---------------- phase 2: energies + softmax ----------------
            q3 = q_sb.rearrange("p (h w) -> p h w", w=W)
            k3 = k_sb.rearrange("p (h w) -> p h w", w=W)
            attH3 = attH.rearrange("p (w h) -> p w h", w=W)
            attW3 = attW.rearrange("p (h w) -> p h w", h=H)
            with tc.tile_pool(name="epsum", bufs=3, space="PSUM") as epp:
                for i in range(H):
                    # E_H^T[j,h] at w=i:  lhsT=k[:, :, i] (j on free), rhs=q[:, :, i]
                    eh = epp.tile([H, H], F32, tag="eh")
                    _mm(nc, eh, k3[:, :, i], q3[:, :, i], True, False)
                    _mm(nc, eh, id16[:H, :H], nid, False, True)
                    nc.scalar.activation(
                        attH3[:, i, :], eh, mybir.ActivationFunctionType.Exp
                    )
                    # E_W^T[j,w] at h=i:  lhsT=k[:, i, :], rhs=q[:, i, :]
                    ew = epp.tile([H, W], F32, tag="ew")
                    _mm(nc, ew, k3[:, i, :], q3[:, i, :], True, True)
                    nc.scalar.activation(
                        attW3[:, i, :], ew, mybir.ActivationFunctionType.Exp
                    )
                    # denom columns: ones^T reductions over j (partition dim)
                    _mm(
                        nc, shp[:, i : i + 1], attH3[:, i, :], ones16,
                        i == 0, i == H - 1,
                    )
                    _mm(
                        nc, swp[:, i : i + 1], attW3[:, i, :], ones16,
                        i == 0, i == H - 1,
                    )

                # Z = SH + SW^T ; izg = gamma/Z (both orientations)
                nc.scalar.copy(sw_sb, swp)
                swt = epp.tile([H, W], F32, tag="eh")
                nc.tensor.transpose(swt, sw_sb, id32[:H, :H])
                nc.scalar.copy(sh_sb, shp)
                nc.vector.tensor_add(z_sb, sh_sb, swt)
                nc.vector.reciprocal(r_sb, z_sb)
                nc.vector.tensor_scalar_mul(izg, r_sb, gcol)
                izgtp = epp.tile([W, H], F32, tag="ew")
                nc.tensor.transpose(izgtp, izg, id32[:H, :H])
                nc.scalar.copy(izgT, izgtp)

            _qk_cm.__exit__(None, None, None)

            # ---------------- phase 3: attention @ V ----------------
            vt_str = vt_d.rearrange("(j w) c -> w j c", w=W)
            ob_str = ob_d.rearrange("(h w) c -> w h c", w=W)
            _acc_cm = tc.tile_pool(name="acc", bufs=1)
            accp = _acc_cm.__enter__()
            accA = accp.tile([W, H * C], BF16, tag="accA")  # [w, (h,c)]
            accA3 = accA.rearrange("p (h c) -> p h c", c=C)
            G = 4
            va_r = vt_d.rearrange("(g i j) c -> g j i c", i=G, j=W)
            vb_r = vt_d.rearrange("(j g w) c -> g j w c", g=H // G, w=G)
            obw_r = ob_d.rearrange("(h g w) c -> g h w c", g=H // G, w=G)
            with (
                tc.tile_pool(name="vblk", bufs=3) as vbp,
                tc.tile_pool(name="ostage", bufs=3) as osp,
                tc.tile_pool(name="opsum", bufs=3, space="PSUM") as opp,
            ):
                for g in range(H // G):
                    va = vbp.tile([H, G, C], BF16, tag="va")
                    nc.sync.dma_start(out=va, in_=va_r[g])
                    vb = vbp.tile([H, G, C], BF16, tag="vb")
                    nc.sync.dma_start(out=vb, in_=vb_r[g])
                    obs = osp.tile([H, G, C], BF16, tag="ob")
                    for i2 in range(G):
                        i = g * G + i2
                        # pass A (row attention, fixed h=i): out[w,c] -> accA
                        oap = opp.tile([W, C], F32, tag="oap")
                        _mm(nc, oap, attW3[:, i, :], va[:, i2, :], True, True)
                        nc.vector.tensor_scalar_mul(
                            accA3[:, i, :], oap, izgT[:, i : i + 1]
                        )
                        # pass B (col attention, fixed w=i): out[h,c] -> DRAM
                        obp = opp.tile([H, C], F32, tag="obp")
                        _mm(nc, obp, attH3[:, i, :], vb[:, i2, :], True, True)
                        nc.scalar.activation(
                            obs[:, i2, :], obp,
                            mybir.ActivationFunctionType.Identity,
                            scale=izg[:, i : i + 1],
                        )
                    nc.scalar.dma_start(out=obw_r[g], in_=obs)

            # ---------------- phase 4: combine, transpose, +x ----------------
            HQ = 4  # h-rows per group
            FW = HQ * W  # 384 output columns per tile
            with (
                tc.tile_pool(name="fin", bufs=3) as fp_,
                tc.tile_pool(name="fsum", bufs=6) as fsp,
                tc.tile_pool(name="fout", bufs=3) as fop,
                tc.tile_pool(name="fpsum", bufs=4, space="PSUM") as fpp,
            ):
                tb_r = ob_d.rearrange("(g i w) c -> g w i c", i=HQ, w=W)
                x4_r = x_d.rearrange("(cc p) n -> p cc n", p=128)
                o4_r = out_d.rearrange("(cc p) n -> p cc n", p=128)
                for hq in range(H // HQ):
                    nsl = bass.ts(hq, FW)
                    tb = fp_.tile([W, HQ, C], BF16, tag="tb")
                    nc.sync.dma_start(out=tb, in_=tb_r[hq])
                    xt = fop.tile([128, CCH, FW], BF16, tag="xt4")
                    nc.sync.dma_start(out=xt, in_=x4_r[:, :, nsl])
                    sus = []
                    for i2 in range(HQ):
                        h = hq * HQ + i2
                        su = fsp.tile([W, C], BF16, tag="su")
                        nc.vector.tensor_add(su, accA3[:, h, :], tb[:, i2, :])
                        sus.append(su)
                    ot = fop.tile([128, CCH, FW], BF16, tag="ot")
                    for cc in range(CCH):
                        csl = bass.ts(cc, 128)
                        op = fpp.tile([128, FW], BF16, tag="op")
                        for i2 in range(HQ):
                            nc.tensor.matmul(
                                op[:, bass.ts(i2, W)], sus[i2][:, csl],
                                id16[:W, :W], is_transpose=True,
                                start=(i2 == 0), stop=(i2 == HQ - 1),
                            )
                        xb = fop.tile([128, FW], BF16, tag="xb4")
                        nc.scalar.activation(
                            xb, op, mybir.ActivationFunctionType.Identity,
                            bias=gb_sb[:, cc : cc + 1], scale=1.0,
                        )
                        nc.vector.tensor_add(ot[:, cc, :], xb, xt[:, cc, :])
                    nc.scalar.dma_start(out=o4_r[:, :, nsl], in_=ot)
            _acc_cm.__exit__(None, None, None)

    nc.compile()
    return nc


_NC_CACHE = {}


def _get_nc():
    if "nc" not in _NC_CACHE:
        _NC_CACHE["nc"] = build_kernel()
    return _NC_CACHE["nc"]


def make_in_maps(x, Wq, bq, Wk, bk, Wv, bv, gamma):
    import ml_dtypes

    bf16 = ml_dtypes.bfloat16
    x = np.asarray(x, np.float32).astype(bf16)
    gamma = np.asarray(gamma, np.float32)
    wqT = np.ascontiguousarray(np.asarray(Wq, np.float32).T).astype(bf16)
    wkT = np.ascontiguousarray(np.asarray(Wk, np.float32).T).astype(bf16)
    wvT = np.ascontiguousarray(np.asarray(Wv, np.float32).T).astype(bf16)

    shared = {
        "wqT": wqT,
        "wkT": wkT,
        "wvT": wvT,
        "bq": np.asarray(bq, np.float32).reshape(CQ, 1),
        "bk": np.asarray(bk, np.float32).reshape(CQ, 1),
        "gb": (gamma[0] * np.asarray(bv, np.float32)).reshape(C, 1),
        "gcol": np.full((H, 1), gamma[0], np.float32),
        "id32": np.eye(128, dtype=np.float32),
        "id16": np.eye(128, dtype=bf16),
        "negid": (NEG * np.eye(H)).astype(bf16),
    }
    return [
        {**shared, "x": np.ascontiguousarray(x[b].reshape(C, N))} for b in range(B)
    ]


def kernel(x, Wq, bq, Wk, bk, Wv, bv, gamma, _trace=False):
    nc = _get_nc()
    in_maps = make_in_maps(x, Wq, bq, Wk, bk, Wv, bv, gamma)
    res = run_bass_kernel_spmd(nc, in_maps, list(range(B)), trace=_trace)
    out = np.stack([res.results[b]["out"].reshape(C, H, W) for b in range(B)])
    return out.astype(np.float32)

